# revision 3
# baseline (speedup 1.0000x reference)
"""Trainium2 Bass kernel for nn_CrossAttentionFusion (cross-attention with QK-LayerNorm).

Sharding: data-parallel — batch b -> NeuronCore b (B=8, 8 cores), no collectives.

Per-core pipeline (batch b):
  host stages query[b].T and key[b].T (so contraction dims land on SBUF partitions)
  q = query @ Wq     (fp32r matmuls, PSUM fp32 accumulation)
  k = key @ Wk, v = key @ Wv
  per-head-dim LayerNorm on q,k (bn_stats/bn_aggr + tensor_scalar), then PE-transpose
  per head:
    path A (natural [i,j] scores): S, exp via ACT (accum_out row sums), P = exp * recip,
      DMA P -> attn output; recips kept
    path B (transposed [j,i] scores): S^T, exp, O^T = V^T-matmul-accumulated, scaled by
      recip broadcast (DRAM step-0 broadcast), written to DRAM scratch
  out = (O @ Wo): contraction over H from O^T scratch tiles, + bo -> out
"""

import numpy as np

import concourse.bacc as bacc
import concourse.bass as bass
import concourse.tile as tile
import concourse.mybir as mybir
from concourse.bass_utils import run_bass_kernel_spmd
from concourse.masks import make_identity

F32 = mybir.dt.float32
F32R = mybir.dt.float32r
AF = mybir.ActivationFunctionType
OP = mybir.AluOpType

B, Lq, Lk = 8, 1024, 576
QD, KD, H, NH, HD = 2048, 1024, 2048, 16, 128
EPS = 1e-5
SCALE = float(HD) ** -0.5
NHG, HH = 4, 4            # head groups x heads per group
IB = Lq // 128            # 8 i-blocks
JBS = [(0, 128), (128, 128), (256, 128), (384, 128), (512, 64)]  # j-blocks of Lk
KCQ = QD // 128           # 16 contraction chunks for q-proj
KCK = KD // 128           # 8 for k/v-proj
IC = 2                    # i-chunks of 512 (path B moving dim)
N_CORES = 8

_CACHE = {}
TRACE = False       # set True (e.g. from test.py) to capture an NTFF profile
LAST_RESULT = None  # BassKernelResults of the most recent run


def _build(flags):
    has_bq, has_bk, has_bv, has_bo, has_gbq, has_gbk = flags
    nc = bacc.Bacc("TRN2", target_bir_lowering=False, debug=False, enable_asserts=False)

    qTin = nc.dram_tensor("qTin", [QD, Lq], F32R, kind="ExternalInput").ap()
    kTin = nc.dram_tensor("kTin", [KD, Lk], F32R, kind="ExternalInput").ap()
    Wq = nc.dram_tensor("Wq", [QD, H], F32R, kind="ExternalInput").ap()
    Wk = nc.dram_tensor("Wk", [KD, H], F32R, kind="ExternalInput").ap()
    Wv = nc.dram_tensor("Wv", [KD, H], F32R, kind="ExternalInput").ap()
    Wo = nc.dram_tensor("Wo", [H, QD], F32R, kind="ExternalInput").ap()
    bq = nc.dram_tensor("bq", [H], F32, kind="ExternalInput").ap() if has_bq else None
    bk = nc.dram_tensor("bk", [H], F32, kind="ExternalInput").ap() if has_bk else None
    bv = nc.dram_tensor("bv", [H], F32, kind="ExternalInput").ap() if has_bv else None
    bo = nc.dram_tensor("bo", [QD], F32, kind="ExternalInput").ap() if has_bo else None
    if has_gbq:
        gq = nc.dram_tensor("gq", [HD], F32, kind="ExternalInput").ap()
        betaq = nc.dram_tensor("betaq", [HD], F32, kind="ExternalInput").ap()
    if has_gbk:
        gk = nc.dram_tensor("gk", [HD], F32, kind="ExternalInput").ap()
        betak = nc.dram_tensor("betak", [HD], F32, kind="ExternalInput").ap()
    out = nc.dram_tensor("out", [Lq, QD], F32, kind="ExternalOutput").ap()
    attn = nc.dram_tensor("attn", [NH, Lq, Lk], F32, kind="ExternalOutput").ap()

    def bcast_dram_row(dst, src_ap, n):
        # broadcast a contiguous DRAM row [n] across all 128 partitions of dst
        src = bass.AP(tensor=src_ap.tensor, offset=src_ap.offset, ap=[[0, 128], [1, n]])
        nc.gpsimd.dma_start(out=dst, in_=src)

    with tile.TileContext(nc) as tc:
        from contextlib import ExitStack
        with ExitStack() as ctx:
            pc = ctx.enter_context(tc.tile_pool(name="pc", bufs=1))
            pw = ctx.enter_context(tc.tile_pool(name="pw", bufs=3))
            pqt = ctx.enter_context(tc.tile_pool(name="pqt", bufs=1))
            pkt = ctx.enter_context(tc.tile_pool(name="pkt", bufs=1))
            pv = ctx.enter_context(tc.tile_pool(name="pv", bufs=1))
            pe = ctx.enter_context(tc.tile_pool(name="pe", bufs=1))
            pt = ctx.enter_context(tc.tile_pool(name="pt", bufs=4))
            pst = ctx.enter_context(tc.tile_pool(name="pst", bufs=8))
            pa = ctx.enter_context(tc.tile_pool(name="pa", bufs=3))
            pp = ctx.enter_context(tc.tile_pool(name="pp", bufs=8, space="PSUM"))
            dr = ctx.enter_context(tc.tile_pool(name="dr", bufs=1, space="DRAM"))

            ident = pc.tile([128, 128], F32, tag="ident")
            make_identity(nc, ident)
            eps_t = pc.tile([128, 1], F32, tag="eps")
            nc.vector.memset(eps_t, EPS)
            if has_gbq:
                gqb = pc.tile([128, HD], F32, tag="gqb")
                betaqb = pc.tile([128, HD], F32, tag="betaqb")
                bcast_dram_row(gqb, gq, HD)
                bcast_dram_row(betaqb, betaq, HD)
            if has_gbk:
                gkb = pc.tile([128, HD], F32, tag="gkb")
                betakb = pc.tile([128, HD], F32, tag="betakb")
                bcast_dram_row(gkb, gk, HD)
                bcast_dram_row(betakb, betak, HD)

            otd = [dr.tile([128, Lq], F32R, tag=f"otd{h}", name=f"otd{h}") for h in range(NH)]

            with tc.tile_pool(name="pin", bufs=1) as pin:
                qtin = []
                for kc in range(KCQ):
                    t = pin.tile([128, Lq], F32R, tag=f"qtin{kc}")
                    nc.sync.dma_start(out=t, in_=qTin[kc * 128:(kc + 1) * 128, :])
                    qtin.append(t)
                ktin = []
                for kc in range(KCK):
                    t = pin.tile([128, Lk], F32R, tag=f"ktin{kc}")
                    nc.sync.dma_start(out=t, in_=kTin[kc * 128:(kc + 1) * 128, :])
                    ktin.append(t)

                def layernorm_group(src, jp, dst, gb, bb):
                    # src: [jp, 128] (PSUM) -> dst [jp, 128] fp32, LN over free dim
                    stats = pst.tile([128, 6], F32, tag="stats")
                    mv = pst.tile([128, 2], F32, tag="mv")
                    nc.vector.bn_stats(out=stats[:jp], in_=src)
                    nc.vector.bn_aggr(out=mv[:jp], in_=stats[:jp])
                    rstd = pst.tile([128, 1], F32, tag="rstd")
                    nc.scalar.activation(out=rstd[:jp], in_=mv[:jp, 1:2], func=AF.Sqrt,
                                         bias=eps_t[:jp], scale=1.0)
                    nc.vector.reciprocal(out=rstd[:jp], in_=rstd[:jp])
                    nc.vector.tensor_scalar(out=dst[:jp], in0=src, scalar1=mv[:jp, 0:1],
                                            scalar2=rstd[:jp], op0=OP.subtract, op1=OP.mult)
                    if gb is not None:
                        nc.vector.tensor_mul(out=dst[:jp], in0=dst[:jp], in1=gb[:jp])
                        nc.vector.tensor_add(out=dst[:jp], in0=dst[:jp], in1=bb[:jp])

                for hg in range(NHG):
                    c0 = hg * 512  # H column offset of this head group

                    # ---------------- q projection (kc-outer, 8 PSUM banks) ----------
                    ps_q = [pp.tile([128, 512], F32, tag="pp", name=f"psq{i}") for i in range(IB)]
                    for kc in range(KCQ):
                        wq_t = pw.tile([128, 512], F32R, tag="wq")
                        nc.sync.dma_start(out=wq_t, in_=Wq[kc * 128:(kc + 1) * 128, c0:c0 + 512])
                        for ib in range(IB):
                            nc.tensor.matmul(ps_q[ib], qtin[kc][:, ib * 128:(ib + 1) * 128],
                                             wq_t, start=(kc == 0), stop=(kc == KCQ - 1))
                    qT = [pqt.tile([128, Lq], F32R, tag=f"qT{hh}", name=f"qT{hh}") for hh in range(HH)]
                    for ib in range(IB):
                        if has_bq:
                            bqb = pw.tile([128, 512], F32, tag="bqb")
                            bcast_dram_row(bqb, bq[c0:c0 + 512], 512)
                            nc.vector.tensor_add(out=ps_q[ib], in0=ps_q[ib], in1=bqb)
                        for hh in range(HH):
                            qln = pt.tile([128, 128], F32, tag="qln")
                            layernorm_group(ps_q[ib][:, hh * 128:(hh + 1) * 128], 128, qln,
                                            gqb if has_gbq else None,
                                            betaqb if has_gbq else None)
                            ps_t = pp.tile([128, 512], F32, tag="pp")
                            nc.tensor.transpose(ps_t[:, :128], qln, ident)
                            nc.scalar.copy(out=qT[hh][:, ib * 128:(ib + 1) * 128],
                                           in_=ps_t[:, :128])

                    # ---------------- k projection ----------------------------------
                    ps_k = [pp.tile([128, 512], F32, tag="pp", name=f"psk{i}") for i in range(len(JBS))]
                    for kc in range(KCK):
                        wk_t = pw.tile([128, 512], F32R, tag="wk")
                        nc.sync.dma_start(out=wk_t, in_=Wk[kc * 128:(kc + 1) * 128, c0:c0 + 512])
                        for jb, (j0, jp) in enumerate(JBS):
                            nc.tensor.matmul(ps_k[jb][:jp], ktin[kc][:, j0:j0 + jp],
                                             wk_t, start=(kc == 0), stop=(kc == KCK - 1))
                    kT = [pkt.tile([128, Lk], F32R, tag=f"kT{hh}", name=f"kT{hh}") for hh in range(HH)]
                    for jb, (j0, jp) in enumerate(JBS):
                        if has_bk:
                            bkb = pw.tile([128, 512], F32, tag="bkb")
                            bcast_dram_row(bkb, bk[c0:c0 + 512], 512)
                            nc.vector.tensor_add(out=ps_k[jb][:jp], in0=ps_k[jb][:jp], in1=bkb[:jp])
                        for hh in range(HH):
                            kln = pt.tile([128, 128], F32, tag="kln")
                            layernorm_group(ps_k[jb][:jp, hh * 128:(hh + 1) * 128], jp, kln,
                                            gkb if has_gbk else None,
                                            betakb if has_gbk else None)
                            ps_t = pp.tile([128, 512], F32, tag="pp")
                            nc.tensor.transpose(ps_t[:, :jp], kln[:jp], ident[:jp, :jp])
                            nc.scalar.copy(out=kT[hh][:, j0:j0 + jp], in_=ps_t[:, :jp])

                    # ---------------- v projection ----------------------------------
                    ps_v = [pp.tile([128, 512], F32, tag="pp", name=f"psv{i}") for i in range(len(JBS))]
                    for kc in range(KCK):
                        wv_t = pw.tile([128, 512], F32R, tag="wv")
                        nc.sync.dma_start(out=wv_t, in_=Wv[kc * 128:(kc + 1) * 128, c0:c0 + 512])
                        for jb, (j0, jp) in enumerate(JBS):
                            nc.tensor.matmul(ps_v[jb][:jp], ktin[kc][:, j0:j0 + jp],
                                             wv_t, start=(kc == 0), stop=(kc == KCK - 1))
                    vt = [pv.tile([128, 512], F32R, tag=f"v{jb}", name=f"v{jb}") for jb in range(len(JBS))]
                    for jb, (j0, jp) in enumerate(JBS):
                        if has_bv:
                            bvb = pw.tile([128, 512], F32, tag="bvb")
                            bcast_dram_row(bvb, bv[c0:c0 + 512], 512)
                            nc.vector.tensor_add(out=ps_v[jb][:jp], in0=ps_v[jb][:jp], in1=bvb[:jp])
                        nc.scalar.copy(out=vt[jb][:jp], in_=ps_v[jb][:jp])

                    # ---------------- attention, 4 heads -----------------------------
                    for hh in range(HH):
                        h = hg * HH + hh
                        recips = pa.tile([128, IB], F32, tag="recips")
                        # path A: natural-layout scores -> softmax -> attn output
                        for ib in range(IB):
                            i0 = ib * 128
                            ps_s1 = pp.tile([128, 512], F32, tag="pp")
                            nc.tensor.matmul(ps_s1, qT[hh][:, i0:i0 + 128], kT[hh][:, 0:512],
                                             start=True, stop=True)
                            ps_s2 = pp.tile([128, 512], F32, tag="pp")
                            nc.tensor.matmul(ps_s2[:, :64], qT[hh][:, i0:i0 + 128],
                                             kT[hh][:, 512:576], start=True, stop=True)
                            pout = pa.tile([128, Lk], F32, tag="pout")
                            sum1 = pst.tile([128, 1], F32, tag="sum1")
                            sum2 = pst.tile([128, 1], F32, tag="sum2")
                            nc.scalar.activation(out=pout[:, 0:512], in_=ps_s1, func=AF.Exp,
                                                 scale=SCALE, accum_out=sum1)
                            nc.scalar.activation(out=pout[:, 512:576], in_=ps_s2[:, :64],
                                                 func=AF.Exp, scale=SCALE, accum_out=sum2)
                            nc.vector.tensor_add(out=sum1, in0=sum1, in1=sum2)
                            nc.vector.reciprocal(out=recips[:, ib:ib + 1], in_=sum1)
                            nc.vector.tensor_scalar_mul(out=pout, in0=pout,
                                                        scalar1=recips[:, ib:ib + 1])
                            nc.sync.dma_start(out=attn[h, i0:i0 + 128, :], in_=pout)
                        # broadcast recips -> [128, Lq] via transpose + DRAM roundtrip
                        ps_rt = pp.tile([128, 512], F32, tag="pp")
                        nc.tensor.transpose(ps_rt[:IB, :128], recips, ident)
                        rrow = pa.tile([IB, 128], F32, tag="rrow")
                        nc.scalar.copy(out=rrow, in_=ps_rt[:IB, :128])
                        recd = dr.tile([IB, 128], F32, tag="recd")
                        nc.sync.dma_start(out=recd, in_=rrow)
                        rb = pa.tile([128, Lq], F32, tag="rb")
                        bcast_dram_row(rb, recd, Lq)
                        # path B: transposed scores -> O^T accumulate
                        exs = [pe.tile([128, Lq], F32R, tag=f"ex{jb}", name=f"ex{jb}") for jb in range(len(JBS))]
                        for jb, (j0, jp) in enumerate(JBS):
                            for ic in range(IC):
                                ps_st = pp.tile([128, 512], F32, tag="pp")
                                nc.tensor.matmul(ps_st[:jp], kT[hh][:, j0:j0 + jp],
                                                 qT[hh][:, ic * 512:(ic + 1) * 512],
                                                 start=True, stop=True)
                                nc.scalar.activation(out=exs[jb][:jp, ic * 512:(ic + 1) * 512],
                                                     in_=ps_st[:jp], func=AF.Exp, scale=SCALE)
                        ot = pa.tile([128, Lq], F32R, tag="ot")
                        for ic in range(IC):
                            ps_ot = pp.tile([128, 512], F32, tag="pp")
                            for jb, (j0, jp) in enumerate(JBS):
                                nc.tensor.matmul(ps_ot, vt[jb][:jp, hh * 128:(hh + 1) * 128],
                                                 exs[jb][:jp, ic * 512:(ic + 1) * 512],
                                                 start=(jb == 0), stop=(jb == len(JBS) - 1))
                            nc.vector.tensor_mul(out=ot[:, ic * 512:(ic + 1) * 512],
                                                 in0=ps_ot, in1=rb[:, ic * 512:(ic + 1) * 512])
                        nc.sync.dma_start(out=otd[h], in_=ot)

            # ---------------- output projection (contraction over H) ---------------
            with tc.tile_pool(name="po", bufs=1) as po:
                ott = []
                for hc in range(NH):
                    t = po.tile([128, Lq], F32R, tag=f"ott{hc}")
                    nc.sync.dma_start(out=t, in_=otd[hc])
                    ott.append(t)
                for nbo in range(4):
                    n0 = nbo * 512
                    if has_bo:
                        bob = pw.tile([128, 512], F32, tag="bob")
                        bcast_dram_row(bob, bo[n0:n0 + 512], 512)
                    ps_o = [pp.tile([128, 512], F32, tag="pp", name=f"pso{i}") for i in range(IB)]
                    for hc in range(NH):
                        wo_t = pw.tile([128, 512], F32R, tag="wo")
                        nc.sync.dma_start(out=wo_t, in_=Wo[hc * 128:(hc + 1) * 128, n0:n0 + 512])
                        for ib in range(IB):
                            nc.tensor.matmul(ps_o[ib], ott[hc][:, ib * 128:(ib + 1) * 128],
                                             wo_t, start=(hc == 0), stop=(hc == NH - 1))
                    for ib in range(IB):
                        osb = pa.tile([128, 512], F32, tag="osb")
                        if has_bo:
                            nc.vector.tensor_add(out=osb, in0=ps_o[ib], in1=bob)
                        else:
                            nc.scalar.copy(out=osb, in_=ps_o[ib])
                        nc.sync.dma_start(out=out[ib * 128:(ib + 1) * 128, n0:n0 + 512], in_=osb)

    nc.compile()
    return nc


def kernel(query, key, Wq, bq, Wk, bk, Wv, bv, Wo, bo, gq, betaq, gk, betak):
    query = np.ascontiguousarray(np.asarray(query, dtype=np.float32))
    key = np.ascontiguousarray(np.asarray(key, dtype=np.float32))
    Wq = np.ascontiguousarray(np.asarray(Wq, dtype=np.float32))
    Wk = np.ascontiguousarray(np.asarray(Wk, dtype=np.float32))
    Wv = np.ascontiguousarray(np.asarray(Wv, dtype=np.float32))
    Wo = np.ascontiguousarray(np.asarray(Wo, dtype=np.float32))
    bq, bk, bv, bo = (np.asarray(x, dtype=np.float32) for x in (bq, bk, bv, bo))
    gq, betaq, gk, betak = (np.asarray(x, dtype=np.float32) for x in (gq, betaq, gk, betak))

    flags = (
        bool(np.any(bq)), bool(np.any(bk)), bool(np.any(bv)), bool(np.any(bo)),
        bool(np.any(gq != 1.0) or np.any(betaq)),
        bool(np.any(gk != 1.0) or np.any(betak)),
    )
    if flags not in _CACHE:
        _CACHE[flags] = _build(flags)
    nc = _CACHE[flags]

    has_bq, has_bk, has_bv, has_bo, has_gbq, has_gbk = flags
    in_maps = []
    for b in range(N_CORES):
        m = {
            "qTin": np.ascontiguousarray(query[b].T),
            "kTin": np.ascontiguousarray(key[b].T),
            "Wq": Wq, "Wk": Wk, "Wv": Wv, "Wo": Wo,
        }
        if has_bq:
            m["bq"] = bq
        if has_bk:
            m["bk"] = bk
        if has_bv:
            m["bv"] = bv
        if has_bo:
            m["bo"] = bo
        if has_gbq:
            m["gq"] = gq
            m["betaq"] = betaq
        if has_gbk:
            m["gk"] = gk
            m["betak"] = betak
        in_maps.append(m)

    res = run_bass_kernel_spmd(nc, in_maps, list(range(N_CORES)), trace=TRACE)
    globals()["LAST_RESULT"] = res
    out = np.stack([res.results[b]["out"] for b in range(N_CORES)])
    attn = np.stack([res.results[b]["attn"] for b in range(N_CORES)])
    return out, attn
